# revision 22
# baseline (speedup 1.0000x reference)
"""MinGRU Trainium2 kernel (nn_MinGRU_60421599920446) — v5.

Math (per batch row):
    vz[s,h] = x[s,:] @ w_z^T + bz      vh[s,h] = x[s,:] @ w_h^T + bh
    z = sigmoid(vz); h_t = (1-z_t)*h_{t-1} + z_t*vh_t   (scan over s)

Strategy: data-parallel over batch, 1 row per NeuronCore (8 cores).

Key ideas vs the v1 baseline:
  * Host pre-transposes and pre-casts x -> xT bf16 [D, S] per core, and the
    kernel emits the output transposed ([H, S] fp16) which the host
    transposes back. Removes ALL PE transposes + PSUM->SBUF transpose
    copies, halves HBM traffic.
  * h = a*h + b (a = 1-z) forgets at E[ln a] ~ -0.8/step, so a chunk scan
    seeded with 0 and warmed up for W=64 steps matches the true state to
    ~e^-50: chunk scans are INDEPENDENT (no carry chain), so the serial
    scan tail can be shortened with smaller final chunks.
  * z/a/b/h fp16 (scan state is fp32 internally); matmul domain bf16.
  * Engine balance (DVE serial scan ~2.1 cyc/elem is the bottleneck):
    ACT: z = sigmoid(vz+bz), vt = vh+bh (the only PSUM readers -> PE's
    PSUM recycles at ACT speed, keeping the PE HAM-warm), plus a share of
    a = sigmoid(-vz-bz). DVE: b = z*vt (packed f16 2x), rest of a = 1-z,
    and all scans (Pool can't scan, and Pool SBUF traffic slows DVE).
  * Scans are emitted interleaved between the next chunk's blocks so the
    DVE FIFO never idles and the end-of-kernel tail is one small scan.
"""

import numpy as np
from contextlib import ExitStack

B, S, D, H = 8, 8192, 256, 256
N_CORES = 8

W = 48            # scan warmup columns (chunks > 0)
PB = 1024         # PSUM block columns (1024 = 2 banks per tile)
CHUNKS = (512, 1024, 2048, 2048, 1024, 1024, 512)  # graduated ramp+tail
A_MAP = "a"       # per-block engine for a: a=ACT sigmoid(-x), v=DVE 1-z

_CACHE = {}


def _build(seq_len, chunks=None, w_warm=W, a_map=A_MAP):
    """Build + compile the single-core SPMD Bass program."""
    import concourse.bacc as bacc
    import concourse.tile as tile
    import concourse.mybir as mybir

    dt = mybir.dt
    f32 = dt.float32
    bf16 = dt.bfloat16
    f16 = dt.float16
    AF = mybir.ActivationFunctionType
    OP = mybir.AluOpType

    if chunks is None:
        chunks = CHUNKS if seq_len == S else (512,) * (seq_len // 512)
    assert sum(chunks) == seq_len
    pb = PB
    assert all(cl % min(cl, pb) == 0 for cl in chunks)

    nc = bacc.Bacc("TRN2", target_bir_lowering=False, debug=False)

    xT_d = nc.dram_tensor("xT", [2, 128, seq_len], bf16, kind="ExternalInput").ap()
    wz_d = nc.dram_tensor("wz", [2, 128, H], bf16, kind="ExternalInput").ap()
    wh_d = nc.dram_tensor("wh", [2, 128, H], bf16, kind="ExternalInput").ap()
    # packed per-partition columns: [half m][128][h0, bz, -bz, bh]
    cols_d = nc.dram_tensor("cols", [2, 128, 4], f32, kind="ExternalInput").ap()
    out_d = nc.dram_tensor("outT", [2, 128, seq_len], f16, kind="ExternalOutput").ap()

    with tile.TileContext(nc) as tc, ExitStack() as ctx:
        const = ctx.enter_context(tc.tile_pool(name="const", bufs=1))
        big = ctx.enter_context(tc.tile_pool(name="big", bufs=1))
        xin = ctx.enter_context(tc.tile_pool(name="xin", bufs=3))
        zp = ctx.enter_context(tc.tile_pool(name="z", bufs=4))
        vtp = ctx.enter_context(tc.tile_pool(name="vt", bufs=4))
        hp = ctx.enter_context(tc.tile_pool(name="h", bufs=2))
        vzp = ctx.enter_context(tc.tile_pool(name="vz", bufs=2, space="PSUM"))
        vhp = ctx.enter_context(tc.tile_pool(name="vh", bufs=2, space="PSUM"))

        if chunks is None:
            pass
        nchunk0 = len(chunks)
        starts0 = [sum(chunks[:i]) for i in range(nchunk0)]
        xtiles = {}
        for c in range(min(1, nchunk0)):
            c0, cl = starts0[c], chunks[c]
            lst = []
            for k in range(2):
                t = xin.tile([128, cl], bf16, tag=f"x{k}{cl}", name=f"x{k}")
                nc.sync.dma_start(t[:], xT_d[k, :, c0:c0 + cl])
                lst.append(t)
            xtiles[c] = lst

        cols = []
        for m in range(2):
            t = const.tile([128, 4], f32, tag=f"cols{m}", name=f"cols{m}")
            nc.sync.dma_start(t[:], cols_d[m])
            cols.append(t)
        wz, wh = [], []
        for k in range(2):
            tz = const.tile([128, H], bf16, tag=f"wz{k}", name=f"wz{k}")
            nc.sync.dma_start(tz[:], wz_d[k])
            wz.append(tz)
            th = const.tile([128, H], bf16, tag=f"wh{k}", name=f"wh{k}")
            nc.sync.dma_start(th[:], wh_d[k])
            wh.append(th)
        ones = const.tile([128, pb], f16, tag="ones", name="ones")
        nc.gpsimd.memset(ones[:], 1.0)
        # touch the Sigmoid table early: the lazy ACT_TABLE_LOAD (~1.3us)
        # otherwise lands on the first real z of the pipeline.
        warmz = const.tile([128, 1], f16, tag="warmz", name="warmz")
        nc.scalar.activation(warmz[:], ones[:, 0:1], AF.Sigmoid,
                             bias=0.0, scale=1.0)

        # big per-core a/b tensors [128, S] fp16, written chunkwise, read by
        # scans with a W-column lookback into the previous chunk's tail.
        A = [big.tile([128, seq_len], f16, tag=f"A{m}", name=f"A{m}")
             for m in range(2)]
        Bb = [big.tile([128, seq_len], f16, tag=f"B{m}", name=f"B{m}")
              for m in range(2)]

        nchunk = len(chunks)
        starts = [sum(chunks[:i]) for i in range(nchunk)]
        pending = []   # scans not yet emitted: (c0, c1, m)
        blk = 0        # running pb-block index (for a_map cycling)

        def emit_scan():
            if not pending:
                return
            c0, c1, m = pending.pop(0)
            ht = hp.tile([128, w_warm + (c1 - c0)], f16,
                         tag=f"h{m}{c1 - c0}", name=f"h{m}")
            if c0 == 0:
                nc.vector.tensor_tensor_scan(
                    ht[:, w_warm:], A[m][:, c0:c1], Bb[m][:, c0:c1],
                    cols[m][:, 0:1], op0=OP.mult, op1=OP.add,
                )
            else:
                nc.vector.tensor_tensor_scan(
                    ht[:], A[m][:, c0 - w_warm:c1], Bb[m][:, c0 - w_warm:c1],
                    0.0, op0=OP.mult, op1=OP.add,
                )
            nc.sync.dma_start(out_d[m, :, c0:c1], ht[:, w_warm:])

        for c in range(nchunk):
            c0 = starts[c]
            clen = chunks[c]
            c1 = c0 + clen
            if c in xtiles:
                xt = xtiles[c]
            else:
                xt = []
                for k in range(2):
                    t = xin.tile([128, clen], bf16, tag=f"x{k}{clen}", name=f"x{k}")
                    nc.sync.dma_start(t[:], xT_d[k, :, c0:c1])
                    xt.append(t)

            pbc = min(pb, clen)
            for p in range(clen // pbc):
                for m in range(2):
                    lo = c0 + p * pbc
                    hi = lo + pbc
                    vzt = vzp.tile([128, pb], f32, tag="vz", name=f"vz{m}")
                    vz = vzt[:, :pbc]
                    for k in range(2):
                        for s2 in range(pbc // 512):
                            nc.tensor.matmul(
                                vz[:, s2 * 512:(s2 + 1) * 512],
                                wz[k][:, m * 128:(m + 1) * 128],
                                xt[k][:, p * pbc + s2 * 512: p * pbc + (s2 + 1) * 512],
                                start=(k == 0), stop=(k == 1),
                            )
                    zt = zp.tile([128, pb], f16, tag=f"z{m}", name=f"z{m}")
                    z = zt[:, :pbc]
                    nc.scalar.activation(z, vz, AF.Sigmoid,
                                         bias=cols[m][:, 1:2], scale=1.0)
                    a_eng = "v" if c == 0 else a_map[blk % len(a_map)]
                    if a_eng == "a":
                        nc.scalar.activation(A[m][:, lo:hi], vz, AF.Sigmoid,
                                             bias=cols[m][:, 2:3], scale=-1.0)
                    else:
                        nc.vector.tensor_tensor(
                            A[m][:, lo:hi], ones[:, :pbc], z, OP.subtract)

                    vht = vhp.tile([128, pb], f32, tag="vh", name=f"vh{m}")
                    vh = vht[:, :pbc]
                    for k in range(2):
                        for s2 in range(pbc // 512):
                            nc.tensor.matmul(
                                vh[:, s2 * 512:(s2 + 1) * 512],
                                wh[k][:, m * 128:(m + 1) * 128],
                                xt[k][:, p * pbc + s2 * 512: p * pbc + (s2 + 1) * 512],
                                start=(k == 0), stop=(k == 1),
                            )
                    if c == 0:
                        nc.vector.scalar_tensor_tensor(
                            Bb[m][:, lo:hi], vh, cols[m][:, 3:4], z,
                            op0=OP.add, op1=OP.mult)
                    else:
                        vtt = vtp.tile([128, pb], f16, tag=f"vt{m}", name=f"vt{m}")
                        vt = vtt[:, :pbc]
                        nc.scalar.activation(vt, vh, AF.Identity,
                                             bias=cols[m][:, 3:4], scale=1.0)
                        nc.vector.tensor_tensor(
                            Bb[m][:, lo:hi], z, vt, OP.mult)
                    blk += 1
                # drain deferred scans between blocks: keeps the DVE FIFO
                # busy and prevents a long scan-only tail after the last
                # chunk's a/b are produced.
                emit_scan()
                if len(pending) > 2:
                    emit_scan()

            pending.append((c0, c1, 0))
            pending.append((c0, c1, 1))

        while pending:
            emit_scan()

    nc.compile()
    return nc


def _get(seq_len, chunks=None, w_warm=W, a_map=A_MAP):
    key = (seq_len, chunks, w_warm, a_map)
    if key not in _CACHE:
        _CACHE[key] = _build(seq_len, chunks, w_warm, a_map)
    return _CACHE[key]


def _make_in_maps(x, h0, w_h_w, w_h_b, w_z_w, w_z_b, n_cores=N_CORES):
    import ml_dtypes
    bf16 = ml_dtypes.bfloat16
    seq_len = x.shape[1]
    wz = np.ascontiguousarray(
        np.asarray(w_z_w, np.float32).T.astype(bf16)).reshape(2, 128, H)
    wh = np.ascontiguousarray(
        np.asarray(w_h_w, np.float32).T.astype(bf16)).reshape(2, 128, H)
    bz = np.asarray(w_z_b, np.float32).reshape(2, 128)
    bh = np.asarray(w_h_b, np.float32).reshape(2, 128)
    in_maps = []
    for i in range(n_cores):
        h0c = np.asarray(h0[i, 0], np.float32).reshape(2, 128)
        cols = np.stack([h0c, bz, -bz, bh], axis=-1)  # [2,128,4]
        xT = np.ascontiguousarray(
            np.asarray(x[i], np.float32).T.astype(bf16)).reshape(2, 128, seq_len)
        in_maps.append({
            "xT": xT, "wz": wz, "wh": wh,
            "cols": np.ascontiguousarray(cols),
        })
    return in_maps


def kernel(x, h0, w_h_w, w_h_b, w_z_w, w_z_b):
    from concourse.bass_utils import run_bass_kernel_spmd

    nc = _get(S)
    in_maps = _make_in_maps(x, h0, w_h_w, w_h_b, w_z_w, w_z_b)
    res = run_bass_kernel_spmd(nc, in_maps, list(range(N_CORES)))
    out = np.empty((N_CORES, S, H), dtype=np.float32)
    for i in range(N_CORES):
        hT = np.asarray(res.results[i]["outT"], dtype=np.float32)  # [2,128,S]
        out[i] = hT.reshape(H, S).T
    return out


# revision 23
# speedup vs baseline: 1.0309x; 1.0309x over previous
"""MinGRU Trainium2 kernel (nn_MinGRU_60421599920446) — v5.

Math (per batch row):
    vz[s,h] = x[s,:] @ w_z^T + bz      vh[s,h] = x[s,:] @ w_h^T + bh
    z = sigmoid(vz); h_t = (1-z_t)*h_{t-1} + z_t*vh_t   (scan over s)

Strategy: data-parallel over batch, 1 row per NeuronCore (8 cores).

Key ideas vs the v1 baseline:
  * Host pre-transposes and pre-casts x -> xT bf16 [D, S] per core, and the
    kernel emits the output transposed ([H, S] fp16) which the host
    transposes back. Removes ALL PE transposes + PSUM->SBUF transpose
    copies, halves HBM traffic.
  * h = a*h + b (a = 1-z) forgets at E[ln a] ~ -0.8/step, so a chunk scan
    seeded with 0 and warmed up for W=64 steps matches the true state to
    ~e^-50: chunk scans are INDEPENDENT (no carry chain), so the serial
    scan tail can be shortened with smaller final chunks.
  * z/a/b/h fp16 (scan state is fp32 internally); matmul domain bf16.
  * Engine balance (DVE serial scan ~2.1 cyc/elem is the bottleneck):
    ACT: z = sigmoid(vz+bz), vt = vh+bh (the only PSUM readers -> PE's
    PSUM recycles at ACT speed, keeping the PE HAM-warm), plus a share of
    a = sigmoid(-vz-bz). DVE: b = z*vt (packed f16 2x), rest of a = 1-z,
    and all scans (Pool can't scan, and Pool SBUF traffic slows DVE).
  * Scans are emitted interleaved between the next chunk's blocks so the
    DVE FIFO never idles and the end-of-kernel tail is one small scan.
"""

import numpy as np
from contextlib import ExitStack

B, S, D, H = 8, 8192, 256, 256
N_CORES = 8

W = 48            # scan warmup columns (chunks > 0)
PB = 1024         # PSUM block columns (1024 = 2 banks per tile)
CHUNKS = (512, 1024, 2048, 2048, 1024, 1024, 512)  # graduated ramp+tail
A_MAP = "aav"     # per-block engine for a: a=ACT sigmoid(-x), v=DVE 1-z

_CACHE = {}


def _build(seq_len, chunks=None, w_warm=W, a_map=A_MAP):
    """Build + compile the single-core SPMD Bass program."""
    import concourse.bacc as bacc
    import concourse.tile as tile
    import concourse.mybir as mybir

    dt = mybir.dt
    f32 = dt.float32
    bf16 = dt.bfloat16
    f16 = dt.float16
    AF = mybir.ActivationFunctionType
    OP = mybir.AluOpType

    if chunks is None:
        chunks = CHUNKS if seq_len == S else (512,) * (seq_len // 512)
    assert sum(chunks) == seq_len
    pb = PB
    assert all(cl % min(cl, pb) == 0 for cl in chunks)

    nc = bacc.Bacc("TRN2", target_bir_lowering=False, debug=False)

    xT_d = nc.dram_tensor("xT", [2, 128, seq_len], bf16, kind="ExternalInput").ap()
    wz_d = nc.dram_tensor("wz", [2, 128, H], bf16, kind="ExternalInput").ap()
    wh_d = nc.dram_tensor("wh", [2, 128, H], bf16, kind="ExternalInput").ap()
    # packed per-partition columns: [half m][128][h0, bz, -bz, bh]
    cols_d = nc.dram_tensor("cols", [2, 128, 4], f32, kind="ExternalInput").ap()
    out_d = nc.dram_tensor("outT", [2, 128, seq_len], f16, kind="ExternalOutput").ap()

    with tile.TileContext(nc) as tc, ExitStack() as ctx:
        const = ctx.enter_context(tc.tile_pool(name="const", bufs=1))
        big = ctx.enter_context(tc.tile_pool(name="big", bufs=1))
        xin = ctx.enter_context(tc.tile_pool(name="xin", bufs=3))
        zp = ctx.enter_context(tc.tile_pool(name="z", bufs=4))
        vtp = ctx.enter_context(tc.tile_pool(name="vt", bufs=4))
        hp = ctx.enter_context(tc.tile_pool(name="h", bufs=2))
        vzp = ctx.enter_context(tc.tile_pool(name="vz", bufs=2, space="PSUM"))
        vhp = ctx.enter_context(tc.tile_pool(name="vh", bufs=2, space="PSUM"))

        if chunks is None:
            pass
        nchunk0 = len(chunks)
        starts0 = [sum(chunks[:i]) for i in range(nchunk0)]
        xtiles = {}
        for c in range(min(1, nchunk0)):
            c0, cl = starts0[c], chunks[c]
            lst = []
            for k in range(2):
                t = xin.tile([128, cl], bf16, tag=f"x{k}{cl}", name=f"x{k}")
                nc.sync.dma_start(t[:], xT_d[k, :, c0:c0 + cl])
                lst.append(t)
            xtiles[c] = lst

        cols = []
        for m in range(2):
            t = const.tile([128, 4], f32, tag=f"cols{m}", name=f"cols{m}")
            nc.sync.dma_start(t[:], cols_d[m])
            cols.append(t)
        wz, wh = [], []
        for k in range(2):
            tz = const.tile([128, H], bf16, tag=f"wz{k}", name=f"wz{k}")
            nc.sync.dma_start(tz[:], wz_d[k])
            wz.append(tz)
            th = const.tile([128, H], bf16, tag=f"wh{k}", name=f"wh{k}")
            nc.sync.dma_start(th[:], wh_d[k])
            wh.append(th)
        ones = const.tile([128, pb], f16, tag="ones", name="ones")
        nc.gpsimd.memset(ones[:], 1.0)
        # touch the Sigmoid table early: the lazy ACT_TABLE_LOAD (~1.3us)
        # otherwise lands on the first real z of the pipeline.
        warmz = const.tile([128, 1], f16, tag="warmz", name="warmz")
        nc.scalar.activation(warmz[:], ones[:, 0:1], AF.Sigmoid,
                             bias=0.0, scale=1.0)

        # big per-core a/b tensors [128, S] fp16, written chunkwise, read by
        # scans with a W-column lookback into the previous chunk's tail.
        A = [big.tile([128, seq_len], f16, tag=f"A{m}", name=f"A{m}")
             for m in range(2)]
        Bb = [big.tile([128, seq_len], f16, tag=f"B{m}", name=f"B{m}")
              for m in range(2)]

        nchunk = len(chunks)
        starts = [sum(chunks[:i]) for i in range(nchunk)]
        pending = []   # scans not yet emitted: (c0, c1, m)
        blk = 0        # running pb-block index (for a_map cycling)

        def emit_scan():
            if not pending:
                return
            c0, c1, m = pending.pop(0)
            ht = hp.tile([128, w_warm + (c1 - c0)], f16,
                         tag=f"h{m}{c1 - c0}", name=f"h{m}")
            if c0 == 0:
                nc.vector.tensor_tensor_scan(
                    ht[:, w_warm:], A[m][:, c0:c1], Bb[m][:, c0:c1],
                    cols[m][:, 0:1], op0=OP.mult, op1=OP.add,
                )
            else:
                nc.vector.tensor_tensor_scan(
                    ht[:], A[m][:, c0 - w_warm:c1], Bb[m][:, c0 - w_warm:c1],
                    0.0, op0=OP.mult, op1=OP.add,
                )
            nc.sync.dma_start(out_d[m, :, c0:c1], ht[:, w_warm:])

        for c in range(nchunk):
            c0 = starts[c]
            clen = chunks[c]
            c1 = c0 + clen
            if c in xtiles:
                xt = xtiles[c]
            else:
                xt = []
                for k in range(2):
                    t = xin.tile([128, clen], bf16, tag=f"x{k}{clen}", name=f"x{k}")
                    nc.sync.dma_start(t[:], xT_d[k, :, c0:c1])
                    xt.append(t)

            pbc = min(pb, clen)
            for p in range(clen // pbc):
                for m in range(2):
                    lo = c0 + p * pbc
                    hi = lo + pbc
                    vzt = vzp.tile([128, pb], f32, tag="vz", name=f"vz{m}")
                    vz = vzt[:, :pbc]
                    for k in range(2):
                        for s2 in range(pbc // 512):
                            nc.tensor.matmul(
                                vz[:, s2 * 512:(s2 + 1) * 512],
                                wz[k][:, m * 128:(m + 1) * 128],
                                xt[k][:, p * pbc + s2 * 512: p * pbc + (s2 + 1) * 512],
                                start=(k == 0), stop=(k == 1),
                            )
                    zt = zp.tile([128, pb], f16, tag=f"z{m}", name=f"z{m}")
                    z = zt[:, :pbc]
                    nc.scalar.activation(z, vz, AF.Sigmoid,
                                         bias=cols[m][:, 1:2], scale=1.0)
                    a_eng = "v" if c == 0 else a_map[blk % len(a_map)]
                    if a_eng == "a":
                        nc.scalar.activation(A[m][:, lo:hi], vz, AF.Sigmoid,
                                             bias=cols[m][:, 2:3], scale=-1.0)
                    else:
                        nc.vector.tensor_tensor(
                            A[m][:, lo:hi], ones[:, :pbc], z, OP.subtract)

                    vht = vhp.tile([128, pb], f32, tag="vh", name=f"vh{m}")
                    vh = vht[:, :pbc]
                    for k in range(2):
                        for s2 in range(pbc // 512):
                            nc.tensor.matmul(
                                vh[:, s2 * 512:(s2 + 1) * 512],
                                wh[k][:, m * 128:(m + 1) * 128],
                                xt[k][:, p * pbc + s2 * 512: p * pbc + (s2 + 1) * 512],
                                start=(k == 0), stop=(k == 1),
                            )
                    if c == 0:
                        nc.vector.scalar_tensor_tensor(
                            Bb[m][:, lo:hi], vh, cols[m][:, 3:4], z,
                            op0=OP.add, op1=OP.mult)
                    else:
                        vtt = vtp.tile([128, pb], f16, tag=f"vt{m}", name=f"vt{m}")
                        vt = vtt[:, :pbc]
                        nc.scalar.activation(vt, vh, AF.Identity,
                                             bias=cols[m][:, 3:4], scale=1.0)
                        nc.vector.tensor_tensor(
                            Bb[m][:, lo:hi], z, vt, OP.mult)
                    blk += 1
                # drain deferred scans between blocks: keeps the DVE FIFO
                # busy and prevents a long scan-only tail after the last
                # chunk's a/b are produced.
                emit_scan()
                if len(pending) > 2:
                    emit_scan()

            pending.append((c0, c1, 0))
            pending.append((c0, c1, 1))

        while pending:
            emit_scan()

    nc.compile()
    return nc


def _get(seq_len, chunks=None, w_warm=W, a_map=A_MAP):
    key = (seq_len, chunks, w_warm, a_map)
    if key not in _CACHE:
        _CACHE[key] = _build(seq_len, chunks, w_warm, a_map)
    return _CACHE[key]


def _make_in_maps(x, h0, w_h_w, w_h_b, w_z_w, w_z_b, n_cores=N_CORES):
    import ml_dtypes
    bf16 = ml_dtypes.bfloat16
    seq_len = x.shape[1]
    wz = np.ascontiguousarray(
        np.asarray(w_z_w, np.float32).T.astype(bf16)).reshape(2, 128, H)
    wh = np.ascontiguousarray(
        np.asarray(w_h_w, np.float32).T.astype(bf16)).reshape(2, 128, H)
    bz = np.asarray(w_z_b, np.float32).reshape(2, 128)
    bh = np.asarray(w_h_b, np.float32).reshape(2, 128)
    in_maps = []
    for i in range(n_cores):
        h0c = np.asarray(h0[i, 0], np.float32).reshape(2, 128)
        cols = np.stack([h0c, bz, -bz, bh], axis=-1)  # [2,128,4]
        xT = np.ascontiguousarray(
            np.asarray(x[i], np.float32).T.astype(bf16)).reshape(2, 128, seq_len)
        in_maps.append({
            "xT": xT, "wz": wz, "wh": wh,
            "cols": np.ascontiguousarray(cols),
        })
    return in_maps


def kernel(x, h0, w_h_w, w_h_b, w_z_w, w_z_b):
    from concourse.bass_utils import run_bass_kernel_spmd

    nc = _get(S)
    in_maps = _make_in_maps(x, h0, w_h_w, w_h_b, w_z_w, w_z_b)
    res = run_bass_kernel_spmd(nc, in_maps, list(range(N_CORES)))
    out = np.empty((N_CORES, S, H), dtype=np.float32)
    for i in range(N_CORES):
        hT = np.asarray(res.results[i]["outT"], dtype=np.float32)  # [2,128,S]
        out[i] = hT.reshape(H, S).T
    return out
